# revision 24
# baseline (speedup 1.0000x reference)
"""Cross-attention kernel for Trainium2, sharded across 8 NeuronCores.

Sharding: data-parallel over batch (B=2) x tensor-parallel over head groups
(16 heads -> 4 groups of 4). Core c handles batch c//4, head group c%4.
Each core projects with its 256-wide column shard of Wq/Wk/Wv, runs attention
for its 4 heads, applies its 256-row shard of Wo, and a ReduceScatter over
each batch group of 4 cores sums the partial outputs and hands each core its
512-row slice of the final output.

Matmul-path data is bf16 (converted host-side); softmax statistics, PSUM
accumulation, and the final output stay fp32. The attention scale 1/sqrt(64)
is folded into Wq on the host.
"""

import sys

sys.path.insert(0, "/opt/trn_rl_repo")

import contextlib

import numpy as np

import concourse.bass as bass
import concourse.mybir as mybir
import concourse.tile as tile
from concourse import bacc

F32 = mybir.dt.float32
BF16 = mybir.dt.bfloat16

B = 2
S = 2048  # both Sq and Sk
D = 1024
NCORES = 8
HEADS_PER_CORE = 4
DH = 64
DG = HEADS_PER_CORE * DH  # 256: per-core projection width
TCOL = 512  # token column width for projections / attention moving dim
NTCOL = S // TCOL  # 4
NIC = D // 128  # 8 input-dim chunks
NKB = S // 128  # 16 key blocks
NQB = S // 128  # 16 query blocks
OUT_ROWS = S // 4  # 512 rows of final output per core (ReduceScatter shard)


def build_compute(tc, ins, st):
    """Phases 1-3: everything except the collective. May sit inside a
    timing repeat loop (all state tiles are rewritten every pass)."""
    nc = tc.nc
    q, k, v = ins["q"], ins["k"], ins["v"]
    wq_sb, wk_sb, wv_sb, wo_sb = st["wq_sb"], st["wk_sb"], st["wv_sb"], st["wo_sb"]
    bq_sb, bk_sb = st["bq_sb"], st["bk_sb"]
    QT, KT, VA, OTs = st["QT"], st["KT"], st["VA"], st["OTs"]
    partial, dram2 = st["partial"], st["dram2"]

    # ---- Phase 1: DMA-transpose inputs + projections ----
    with (
        tc.tile_pool(name="tT", bufs=3) as tTp,
        tc.tile_pool(name="ps_p", bufs=4, space="PSUM") as ps_p,
    ):
        for which, src in (("q", q), ("k", k), ("v", v)):
            for tcol in range(NTCOL):
                tT = tTp.tile([128, NIC, TCOL], BF16)
                for ic in range(NIC):
                    nc.sync.dma_start(
                        out=tT[:, ic, :],
                        in_=src[
                            tcol * TCOL : (tcol + 1) * TCOL,
                            ic * 128 : (ic + 1) * 128,
                        ],
                        transpose=True,
                    )
                if which in ("q", "k"):
                    dstT = QT if which == "q" else KT
                    bias = bq_sb if which == "q" else bk_sb
                    w_sb = wq_sb if which == "q" else wk_sb
                    for db in range(2):
                        pp = ps_p.tile([128, TCOL], F32)
                        for ic in range(NIC):
                            nc.tensor.matmul(
                                pp[:],
                                w_sb[:, ic, db * 128 : (db + 1) * 128],
                                tT[:, ic, :],
                                start=(ic == 0),
                                stop=(ic == NIC - 1),
                            )
                        nc.vector.tensor_scalar_add(
                            dstT[:, db, tcol * TCOL : (tcol + 1) * TCOL],
                            pp[:],
                            bias[:, db, :],
                        )
                else:
                    for tb in range(4):
                        pp = ps_p.tile([128, TCOL], F32)
                        for ic in range(NIC):
                            nc.tensor.matmul(
                                pp[:, 0:DG],
                                tT[:, ic, tb * 128 : (tb + 1) * 128],
                                wv_sb[:, ic, :],
                                start=(ic == 0),
                                stop=(ic == NIC - 1),
                            )
                        kb = tcol * 4 + tb
                        nc.vector.tensor_copy(
                            VA[:, kb, :, 0:DH],
                            pp[:, 0:DG].rearrange("p (h d) -> p h d", d=DH),
                        )

    # ---- Phase 2: attention, head pairs on PE row groups ----
    # Pair (2*t2, 2*t2+1): head hh sits at partitions hh*64..hh*64+64 of the
    # QT/KT tiles, so its S^T matmuls run in PE row group hh concurrently
    # with the other head's.
    with (
        tc.tile_pool(name="ps_S", bufs=2, space="PSUM") as psS,
        tc.tile_pool(name="ps_O", bufs=1, space="PSUM") as psO,
        tc.tile_pool(name="PT", bufs=4) as PTp,
        tc.tile_pool(name="rb", bufs=2) as rbp,
        tc.tile_pool(name="dn", bufs=2) as dnp,
    ):
        for t2 in range(2):
            for qh in range(2):  # query half: columns qh*1024 .. +1024
                pO = {
                    (hh, jq): psO.tile(
                        [128, TCOL], F32, name=f"pO{hh}{jq}", tag=f"pO{hh}{jq}"
                    )
                    for hh in range(2)
                    for jq in range(2)
                }
                for kb in range(NKB):
                    pss = [psS.tile([128, 1024], F32, name="ps") for _ in range(2)]
                    for jq in range(2):
                        for hh in range(2):
                            qoff = qh * 1024 + jq * TCOL
                            nc.tensor.matmul(
                                pss[hh][:, jq * TCOL : (jq + 1) * TCOL],
                                KT[hh * 64 : hh * 64 + 64, t2,
                                   kb * 128 : (kb + 1) * 128],
                                QT[hh * 64 : hh * 64 + 64, t2,
                                   qoff : qoff + TCOL],
                                start=True,
                                stop=True,
                            )
                    pts = []
                    for hh in range(2):
                        pt = PTp.tile([128, 1024], BF16, name="pt")
                        nc.scalar.activation(
                            pt[:], pss[hh][:], mybir.ActivationFunctionType.Exp
                        )
                        pts.append(pt)
                    for hh in range(2):
                        for jq in range(2):
                            nc.tensor.matmul(
                                pO[(hh, jq)][0 : DH + 1, :],
                                VA[:, kb, 2 * t2 + hh, :],
                                pts[hh][:, jq * TCOL : (jq + 1) * TCOL],
                                start=(kb == 0),
                                stop=(kb == NKB - 1),
                            )
                # normalize this q-half for both heads of the pair
                for hh in range(2):
                    h = 2 * t2 + hh
                    dn = dnp.tile([65, 1024], F32, name=f"dn{hh}", tag=f"dn{hh}")
                    for jq in range(2):
                        nc.vector.reciprocal(
                            dn[64:65, jq * TCOL : (jq + 1) * TCOL],
                            pO[(hh, jq)][64:65, :],
                        )
                    scr = dram2.tile([1, 1024], F32, name="scr", tag="scr")
                    nc.sync.dma_start(out=scr[:], in_=dn[64:65, :])
                    rb = rbp.tile([64, 1024], F32, name=f"rb{hh}", tag=f"rb{hh}")
                    scr_ap = scr[:]
                    bcast = bass.AP(
                        tensor=scr_ap.tensor,
                        offset=scr_ap.offset,
                        ap=[[0, 64], [1, 1024]],
                    )
                    nc.sync.dma_start(out=rb[:], in_=bcast)
                    for jq in range(2):
                        nc.vector.tensor_mul(
                            OTs[:, h,
                                qh * 1024 + jq * TCOL : qh * 1024 + (jq + 1) * TCOL],
                            pO[(hh, jq)][0:64, :],
                            rb[:, jq * TCOL : (jq + 1) * TCOL],
                        )

    # ---- Phase 3: output projection (row-sharded Wo, heads summed in PSUM) ----
    with (
        tc.tile_pool(name="ps_Z", bufs=2, space="PSUM") as psZ,
        tc.tile_pool(name="zsb", bufs=3) as zp,
    ):
        for qb in range(NQB):
            zz = [
                psZ.tile([128, TCOL], F32, name=f"z{n2}", tag=f"z{n2}")
                for n2 in range(2)
            ]
            for h in range(HEADS_PER_CORE):
                for n2 in range(2):
                    nc.tensor.matmul(
                        zz[n2][:],
                        OTs[:, h, qb * 128 : (qb + 1) * 128],
                        wo_sb[:, h, n2 * TCOL : (n2 + 1) * TCOL],
                        start=(h == 0),
                        stop=(h == HEADS_PER_CORE - 1),
                    )
            zt = zp.tile([128, D], F32)
            for n2 in range(2):
                nc.vector.tensor_copy(zt[:, n2 * TCOL : (n2 + 1) * TCOL], zz[n2][:])
            nc.sync.dma_start(
                out=partial[qb * 128 : (qb + 1) * 128, :], in_=zt[:]
            )


def build_attention_kernel(tc, es, ins, out_ext, loop_n=1, rs_n=1):
    nc = tc.nc
    wq, wk, wv, wo = ins["wq"], ins["wk"], ins["wv"], ins["wo"]
    bq, bk = ins["bq"], ins["bk"]

    wpool = es.enter_context(tc.tile_pool(name="wpool", bufs=1))
    big = es.enter_context(tc.tile_pool(name="big", bufs=1))
    dram = es.enter_context(tc.tile_pool(name="dram", bufs=1, space="DRAM"))
    dram2 = es.enter_context(tc.tile_pool(name="dram2", bufs=2, space="DRAM"))

    # Weights into SBUF.
    wq_sb = wpool.tile([128, NIC, DG], BF16)
    wk_sb = wpool.tile([128, NIC, DG], BF16)
    wv_sb = wpool.tile([128, NIC, DG], BF16)
    nc.sync.dma_start(out=wq_sb[:], in_=wq.rearrange("(c p) d -> p c d", p=128))
    nc.sync.dma_start(out=wk_sb[:], in_=wk.rearrange("(c p) d -> p c d", p=128))
    nc.sync.dma_start(out=wv_sb[:], in_=wv.rearrange("(c p) d -> p c d", p=128))
    wo_sb = wpool.tile([64, HEADS_PER_CORE, D], BF16)
    nc.sync.dma_start(out=wo_sb[:], in_=wo.rearrange("(h p) n -> p h n", p=64))
    bq_sb = wpool.tile([128, 2, 1], F32)
    bk_sb = wpool.tile([128, 2, 1], F32)
    nc.sync.dma_start(out=bq_sb[:], in_=bq.rearrange("(c p) x -> p c x", p=128))
    nc.sync.dma_start(out=bk_sb[:], in_=bk.rearrange("(c p) x -> p c x", p=128))

    # Persistent activations.
    QT = big.tile([128, 2, S], BF16)  # [dim%128, dimblock, tok] = (q @ Wq).T
    KT = big.tile([128, 2, S], BF16)
    VA = big.tile([128, NKB, HEADS_PER_CORE, DH + 1], BF16)  # V + ones col
    OTs = big.tile([64, HEADS_PER_CORE, S], BF16)  # normalized O^T per head
    nc.vector.memset(VA[:, :, :, DH : DH + 1], 1.0)

    partial = dram.tile([S, D], F32)

    st = dict(
        wq_sb=wq_sb, wk_sb=wk_sb, wv_sb=wv_sb, wo_sb=wo_sb,
        bq_sb=bq_sb, bk_sb=bk_sb, QT=QT, KT=KT, VA=VA, OTs=OTs,
        partial=partial, dram2=dram2,
    )

    if loop_n > 1:
        with tc.For_i(0, loop_n, 1):
            build_compute(tc, ins, st)
    else:
        build_compute(tc, ins, st)

    # ---- Phase 4: chunked ReduceScatter (overlaps tail of phase 3) ----
    # Two RS over q-halves; within a batch group rank g gets rows
    # [256g:256g+256] of each half. Host reassembles accordingly.
    if rs_n == 0:
        nc.sync.dma_start(out=out_ext, in_=partial[0:OUT_ROWS, :])
        return
    half_rows = S // 2  # 1024
    shard = half_rows // 4  # 256 rows per rank per chunk
    rs_out = dram.tile([OUT_ROWS, D], F32)
    for _ in range(rs_n):
        for ch in range(2):
            nc.gpsimd.collective_compute(
                "ReduceScatter",
                mybir.AluOpType.add,
                replica_groups=[[0, 1, 2, 3], [4, 5, 6, 7]],
                ins=[partial[ch * half_rows : (ch + 1) * half_rows, :].opt()],
                outs=[rs_out[ch * shard : (ch + 1) * shard, :].opt()],
            )
    nc.sync.dma_start(out=out_ext, in_=rs_out[:])


def build_nc(loop_n=1, rs_n=1):
    nc = bacc.Bacc(
        "TRN2", target_bir_lowering=False, debug=False, num_devices=NCORES
    )
    ins = {}
    for nm in ("q", "k", "v"):
        ins[nm] = nc.dram_tensor(nm, [S, D], BF16, kind="ExternalInput").ap()
    for nm in ("wq", "wk", "wv"):
        ins[nm] = nc.dram_tensor(nm, [D, DG], BF16, kind="ExternalInput").ap()
    ins["wo"] = nc.dram_tensor("wo", [DG, D], BF16, kind="ExternalInput").ap()
    ins["bq"] = nc.dram_tensor("bq", [DG, 1], F32, kind="ExternalInput").ap()
    ins["bk"] = nc.dram_tensor("bk", [DG, 1], F32, kind="ExternalInput").ap()
    out_ext = nc.dram_tensor("out", [OUT_ROWS, D], F32, kind="ExternalOutput").ap()

    from contextlib import ExitStack

    with tile.TileContext(nc) as tc:
        with ExitStack() as es:
            build_attention_kernel(tc, es, ins, out_ext, loop_n=loop_n, rs_n=rs_n)
    nc.compile()
    return nc


def make_in_maps(q, k, v, Wq, bq, Wk, bk, Wv, bv, Wo, bo):
    """Host-side sharding. Attention scale (1/sqrt(64)) is folded into Wq.
    Matmul-path tensors are converted to bf16 on the host."""
    import ml_dtypes

    bf16 = ml_dtypes.bfloat16
    scale = DH**-0.5
    qb16 = [np.ascontiguousarray(q[b]).astype(bf16) for b in range(B)]
    kb16 = [np.ascontiguousarray(k[b]).astype(bf16) for b in range(B)]
    vb16 = [np.ascontiguousarray(v[b]).astype(bf16) for b in range(B)]
    in_maps = []
    for c in range(NCORES):
        b, g = c // 4, c % 4
        cols = slice(g * DG, (g + 1) * DG)
        in_maps.append(
            {
                "q": qb16[b],
                "k": kb16[b],
                "v": vb16[b],
                "wq": np.ascontiguousarray(Wq[:, cols] * scale).astype(bf16),
                "wk": np.ascontiguousarray(Wk[:, cols]).astype(bf16),
                "wv": np.ascontiguousarray(Wv[:, cols]).astype(bf16),
                "wo": np.ascontiguousarray(Wo[cols, :]).astype(bf16),
                "bq": np.ascontiguousarray(
                    (bq[cols] * scale).reshape(DG, 1), dtype=np.float32
                ),
                "bk": np.ascontiguousarray(
                    bk[cols].reshape(DG, 1), dtype=np.float32
                ),
            }
        )
    return in_maps


def assemble_output(results, bv, bo, Wo):
    out = np.empty((B, S, D), np.float32)
    shard = S // 2 // 4  # 256 rows per rank per RS chunk
    for c in range(NCORES):
        b, g = c // 4, c % 4
        r = results[c]["out"]
        out[b, g * shard : (g + 1) * shard, :] = r[0:shard]
        out[b, S // 2 + g * shard : S // 2 + (g + 1) * shard, :] = r[shard:]
    # bv's contribution commutes through softmax-normalized attention and the
    # output projection as a constant row offset; bo is a plain offset.
    bo_eff = np.asarray(bo, np.float64) + np.asarray(bv, np.float64) @ np.asarray(
        Wo, np.float64
    )
    if np.any(bo_eff):
        out += bo_eff[None, None, :].astype(np.float32)
    return out


_NC_CACHE = None


def kernel(q, k, v, Wq, bq, Wk, bk, Wv, bv, Wo, bo):
    global _NC_CACHE
    from concourse.bass_utils import run_bass_kernel_spmd

    args = [
        np.asarray(x, np.float32) for x in (q, k, v, Wq, bq, Wk, bk, Wv, bv, Wo, bo)
    ]
    q, k, v, Wq, bq, Wk, bk, Wv, bv, Wo, bo = args
    if _NC_CACHE is None:
        _NC_CACHE = build_nc()
    nc = _NC_CACHE
    in_maps = make_in_maps(q, k, v, Wq, bq, Wk, bk, Wv, bv, Wo, bo)
    res = run_bass_kernel_spmd(nc, in_maps, core_ids=list(range(NCORES)))
    return assemble_output(res.results, bv, bo, Wo)
